# revision 25
# baseline (speedup 1.0000x reference)
"""Trainium2 Bass kernel for: conv2d(16->64, 3x3, VALID) + bias -> min over
channels -> tanh(tanh()).  Input x [64,16,256,256] f32, output [64,1,254,254].

Strategy (per core, data-parallel over batch: 8 images/core):
  - Conv as matmuls with the x-patch (bf16) as the stationary operand and a
    block-Toeplitz weight matrix (bf16) moving: conv output lands as
    [width-positions (partitions), rows*couts (free)] in f32 PSUM and the
    channel-min is a free-dim reduce.  6-row windows give R=4 output rows
    per 3-matmul (dx) group; TWO consecutive windows pack into ONE 2KB PSUM
    bank ([128, 2, 4, 64] = 512 f32), so every drain op sees a full bank
    and the per-op fixed costs (120 cyc PSUM init on DVE, 172 on ACT)
    amortize over 8 output rows instead of 5.
  - The channel-min drains 33M f32 PSUM elements/core -- more than either
    DVE or ACT alone can move at the PE's pace -- so bank-tiles alternate
    between two pipelines: path A = direct DVE tensor_reduce (~730ns);
    path B = ACT copy PSUM->SBUF bf16 in a cout-half-split layout (~790ns
    ACT), one FLAT single-dim-AP DVE tensor_tensor min fold (flat APs are
    required for the 2x_1p uop to engage; strided views run 1x) and a
    half-size DVE reduce (~640ns DVE total).  OFF_PAT at 13/16 B balances
    DVE ~312us and ACT ~291us under the PE's ~339us; measured exec
    367us vs 439us for the R=5 + PE-transpose-epilogue baseline.
  - Min results land in bf16 staging [128 j, 256 rows]; the epilogue is
    PE-free: ACT double-tanh straight off staging, DMA out in transposed
    [jb, j, row] layout, and the host reassembles with cheap numpy
    transposes -- saving the PE transposes and freeing 2 PSUM banks.
"""

import sys

for _p in ("/opt/trn_rl_repo", "/root/.axon_site/_ro/trn_rl_repo"):
    if _p not in sys.path:
        sys.path.insert(0, _p)

import numpy as np

B, CIN, H, W = 64, 16, 256, 256
COUT, KK = 64, 3
HO, WO = H - 2, W - 2  # 254
N_CORES = 8
B_LOC = B // N_CORES  # 8 images per core

# geometry
WIN_ROWS = 6          # input rows per window
R = WIN_ROWS - KK + 1  # 4 output rows per window
KDIM = (CIN + 1) * WIN_ROWS  # 102 contraction rows (incl. ones channel)
NDIM = R * COUT       # 256 moving free size
MJ = 128              # output width positions per j-block
J0S = (0, WO - MJ)    # j origin per block; cols 126/127 overlap benignly
N_JB = 2
N_WIN = 64            # windows: row0 = 4w for w<63, 250 for w=63
N_PAIR = N_WIN // 2   # two windows share one PSUM bank
_cache = {}


def _row0(w):
    return 4 * w if w < N_WIN - 1 else HO - R  # 250


# Per-(image, jb) drain plan over the 32 bank-pairs, in bank order.
# 'A' banks: direct DVE reduce (~730ns measured).  'B' groups: each bank is
# ACT-copied to a shared SBUF buffer (~791ns ACT), then ONE flat 2x DVE
# fold + ONE reduce cover the whole group, amortizing DVE fixed costs
# (~425ns DVE per bank vs 644 ungrouped).  10 A + 22 B balances
# DVE ~266us vs ACT ~278us per core against the PE's 339us.
# Bank-tile t uses path B (ACT copy + flat DVE fold + reduce) when
# OFF_PAT[t % len] else path A (direct DVE reduce).  Measured per-bank:
# A = DVE ~730ns; B = ACT ~791ns + DVE ~644ns.  13/16 B balances
# DVE ~312us / ACT ~291us under the PE's ~339us.
OFF_PAT = (False, True, True, True, True, True, False, True, True, True,
           True, False, True, True, True, True)
OFF_PAT_12 = (False, True, True, True, False, True, True, True, False, True,
              True, True, False, True, True, True)
OFF_PAT_14 = (False, True, True, True, True, True, True, True, False, True,
              True, True, True, True, True, True)


def _build_wblocks(conv_weight, conv_bias):
    """wblk[dx][rho*17+ci, r*64+co] = W[co,ci,rho-r,dx]; bias on the ones-
    channel row (rho=0, ci=CIN) of dx=0.  Partition order matches the
    [B, H, C, W] host layout of x so the window DMA merges (row, chan)."""
    wblk = np.zeros((KK, KDIM, NDIM), dtype=np.float32)
    for dx in range(KK):
        for ci in range(CIN):
            for rho in range(WIN_ROWS):
                k = rho * (CIN + 1) + ci
                for r in range(R):
                    dy = rho - r
                    if 0 <= dy < KK:
                        wblk[dx, k, r * COUT:(r + 1) * COUT] = conv_weight[:, ci, dy, dx]
    k_bias = CIN  # (rho=0, ci=16)
    for r in range(R):
        wblk[0, k_bias, r * COUT:(r + 1) * COUT] = conv_bias
    return wblk


def _build_nc(reps=1, ablate=()):
    import concourse.bass as bass
    import concourse.bacc as bacc
    import concourse.tile as tile
    from concourse import mybir

    f32 = mybir.dt.float32
    bf16 = mybir.dt.bfloat16

    nc = bacc.Bacc(None)
    # x_aug host layout is [B, H, C, W]: window partitions are (row, chan)
    x_aug = nc.dram_tensor("x_aug", [B_LOC, H, CIN + 1, W], bf16, kind="ExternalInput")
    wblk_d = nc.dram_tensor("wblk", [KK, KDIM, NDIM], bf16, kind="ExternalInput")
    # output in transposed layout [img, jb, j, row]; host reassembles
    y = nc.dram_tensor("y", [B_LOC, N_JB, MJ, HO], f32, kind="ExternalOutput")

    with tile.TileContext(nc) as tc:
        with (
            tc.tile_pool(name="consts", bufs=1) as consts,
            tc.tile_pool(name="wins", bufs=3) as wins,
            tc.tile_pool(name="stage", bufs=4) as stage,
            tc.tile_pool(name="fold", bufs=2) as fold,
            tc.tile_pool(name="outs", bufs=4) as outs,
            tc.tile_pool(name="cpsum", bufs=8, space="PSUM") as cpsum,
        ):
            wblk_s = consts.tile([KDIM, KK, NDIM], bf16)
            nc.sync.dma_start(out=wblk_s[:], in_=wblk_d.rearrange("k d n -> d k n"))

            import contextlib
            loop_ctx = tc.For_i(0, reps, 1) if reps > 1 else contextlib.nullcontext()
            with loop_ctx:
                _emit_body(nc, tc, bass, mybir, ablate, locals())
    nc.finalize()
    return nc


def _emit_body(nc, tc, bass, mybir, ablate, env):
    f32 = env["f32"]
    bf16 = env["bf16"]
    x_aug, y = env["x_aug"], env["y"]
    wblk_s = env["wblk_s"]
    wins, stage, fold, outs = env["wins"], env["stage"], env["fold"], env["outs"]
    cpsum = env["cpsum"]
    CW = (CIN + 1) * W  # elements per image row (all channels)
    MIN = mybir.AluOpType.min

    def _stg_out(stg, p):
        """Staging view [2, 4] for pair p's 8 output rows.  Pairs 0..30 are
        contiguous (rows 8p..8p+7); the last pair overlaps benignly (rows
        248..251 and 250..253 -- row 250/251 written twice, same value)."""
        r0a, r0b = _row0(2 * p), _row0(2 * p + 1)
        return bass.AP(
            tensor=stg.tensor,
            offset=stg.offset + r0a,
            ap=[list(stg.ap[0]), [r0b - r0a, 2], [1, R]],
        )

    def _epilogue(b, stagings):
        for jb in range(N_JB):
            t1 = outs.tile([MJ, HO], f32, name="t1")
            nc.scalar.activation(
                out=t1[:], in_=stagings[jb][:, 0:HO],
                func=mybir.ActivationFunctionType.Tanh,
            )
            t2 = outs.tile([MJ, HO], f32, name="t2")
            nc.scalar.activation(
                out=t2[:], in_=t1[:],
                func=mybir.ActivationFunctionType.Tanh,
            )
            # out-DMA on the SP ring: a DMA occupies its issuing engine's
            # queue for the full transfer, and ACT is drain-critical
            nc.sync.dma_start(out=y[b, jb], in_=t2[:])

    for b in range(B_LOC):
        bigx = wins.tile([KDIM, N_WIN, W], bf16, name="bigx")
        # windows 0..62 (uniform row0 = 4w) in chunked DMAs on the SP ring
        # (ACT's ring would stall the drain copies); w=63 alone.  Image 0
        # only: a small 6-window first chunk so the first matmuls start
        # ~7us earlier (they otherwise gate on a 9us 1.1MB chunk after the
        # 7us NEFF preamble).
        x_b = x_aug[b]
        w_lo = 0
        for nw in ((2, 5, 9, 14, 33) if b == 0 else (21, 21, 21)):
            src = bass.AP(
                tensor=x_b.tensor,
                offset=x_b.offset + 4 * w_lo * CW,
                ap=[[CW, WIN_ROWS], [W, CIN + 1], [4 * CW, nw], [1, W]],
            )
            nc.sync.dma_start(out=bigx[:, w_lo:w_lo + nw, :], in_=src)
            w_lo += nw
        nc.sync.dma_start(
            out=bigx[:, N_WIN - 1, :],
            in_=x_aug[b, HO - R:H, :, :].rearrange("r c w -> (r c) w"),
        )

        def _win(w):
            return bigx[:, w, :]

        stagings = []
        for jb in range(N_JB):
            staging = stage.tile([MJ, 256], bf16, name=f"staging{jb}", tag=f"st{jb}")
            stagings.append(staging)

        def _bank_matmuls(p, j0):
            psum = cpsum.tile([MJ, 2, NDIM], f32, name="psum")
            for u in range(2):
                win = _win(2 * p + u)
                for dx in range(KK):
                    nc.tensor.matmul(
                        out=psum[:, u],
                        lhsT=win[:, j0 + dx:j0 + dx + MJ],
                        rhs=wblk_s[:, dx, :],
                        start=(dx == 0),
                        stop=(dx == KK - 1),
                    )
            return psum

        pat = (OFF_PAT_12 if "x12" in ablate
               else OFF_PAT_14 if "x14" in ablate else OFF_PAT)
        for p in range(N_PAIR):
            for jb in range(N_JB):
                j0 = J0S[jb]
                t = p * N_JB + jb
                psum = _bank_matmuls(p, j0)
                offload = pat[t % len(pat)] and "nooff" not in ablate
                stg_view = _stg_out(stagings[jb], p)
                if offload:
                    # path B: ACT drains the bank as bf16 with cout-halves
                    # split to the outer axis; one flat 2x DVE fold then a
                    # half-size reduce
                    lb = fold.tile([MJ, 2, 2, R, 32], bf16, name="lb")
                    nc.scalar.activation(
                        out=lb.rearrange("p c2 u r c -> p u r c2 c"),
                        in_=psum.rearrange("p u (r c2 c) -> p u r c2 c",
                                           c2=2, c=32),
                        func=mybir.ActivationFunctionType.Copy,
                    )
                    lflat = lb.rearrange("p c2 u r c -> p (c2 u r c)")
                    g = fold.tile([MJ, 2, R, 32], bf16, name="g")
                    nc.vector.tensor_tensor(
                        out=g.rearrange("p u r c -> p (u r c)"),
                        in0=lflat[:, 0:2 * R * 32],
                        in1=lflat[:, 2 * R * 32:4 * R * 32],
                        op=MIN,
                    )
                    nc.vector.tensor_reduce(
                        out=stg_view,
                        in_=g[:],
                        axis=mybir.AxisListType.X,
                        op=MIN,
                    )
                else:
                    # path A: direct DVE reduce from the full PSUM bank
                    nc.vector.tensor_reduce(
                        out=stg_view,
                        in_=psum.rearrange("p u (r c) -> p u r c", c=COUT),
                        axis=mybir.AxisListType.X,
                        op=MIN,
                    )
        _epilogue(b, stagings)


def _get_compiled(reps=1, ablate=()):
    key = ("nc", reps, tuple(ablate))
    if key not in _cache:
        _cache[key] = _build_nc(reps, ablate)
    return _cache[key]


def _to_bf16(a):
    import ml_dtypes
    return np.asarray(a, dtype=np.float32).astype(ml_dtypes.bfloat16)


def make_in_maps(x, conv_weight, conv_bias):
    x = np.asarray(x, dtype=np.float32)
    x_aug = np.empty((B, H, CIN + 1, W), dtype=np.float32)
    x_aug[:, :, :CIN] = x.transpose(0, 2, 1, 3)
    x_aug[:, :, CIN] = 1.0
    x_aug = _to_bf16(x_aug)
    wblk = _to_bf16(_build_wblocks(
        np.asarray(conv_weight, dtype=np.float32),
        np.asarray(conv_bias, dtype=np.float32)))
    return [
        {
            "x_aug": np.ascontiguousarray(x_aug[c * B_LOC:(c + 1) * B_LOC]),
            "wblk": wblk,
        }
        for c in range(N_CORES)
    ]


def kernel(x, conv_weight, conv_bias):
    from concourse.bass_utils import run_bass_kernel_spmd

    nc = _get_compiled()
    in_maps = make_in_maps(x, conv_weight, conv_bias)
    res = run_bass_kernel_spmd(nc, in_maps, core_ids=list(range(N_CORES)))
    out = np.empty((B, 1, HO, WO), dtype=np.float32)
    for c in range(N_CORES):
        yc = res.results[c]["y"]  # [B_LOC, 2, MJ, HO]
        blk = out[c * B_LOC:(c + 1) * B_LOC, 0]
        blk[:, :, 0:MJ] = yc[:, 0].transpose(0, 2, 1)
        blk[:, :, J0S[1]:WO] = yc[:, 1].transpose(0, 2, 1)
    return out


# revision 26
# speedup vs baseline: 1.0119x; 1.0119x over previous
"""Trainium2 Bass kernel for: conv2d(16->64, 3x3, VALID) + bias -> min over
channels -> tanh(tanh()).  Input x [64,16,256,256] f32, output [64,1,254,254].

Strategy (per core, data-parallel over batch: 8 images/core):
  - Conv as matmuls with the x-patch (bf16) as the stationary operand and a
    block-Toeplitz weight matrix (bf16) moving: conv output lands as
    [width-positions (partitions), rows*couts (free)] in f32 PSUM and the
    channel-min is a free-dim reduce.  6-row windows give R=4 output rows
    per 3-matmul (dx) group; TWO consecutive windows pack into ONE 2KB PSUM
    bank ([128, 2, 4, 64] = 512 f32), so every drain op sees a full bank
    and the per-op fixed costs (120 cyc PSUM init on DVE, 172 on ACT)
    amortize over 8 output rows instead of 5.
  - The channel-min drains 33M f32 PSUM elements/core -- more than either
    DVE or ACT alone can move at the PE's pace -- so bank-tiles alternate
    between two pipelines: path A = direct DVE tensor_reduce (~730ns);
    path B = ACT copy PSUM->SBUF bf16 in a cout-half-split layout (~790ns
    ACT), one FLAT single-dim-AP DVE tensor_tensor min fold (flat APs are
    required for the 2x_1p uop to engage; strided views run 1x) and a
    half-size DVE reduce (~640ns DVE total).  OFF_PAT at 13/16 B balances
    DVE ~312us and ACT ~291us under the PE's ~339us; measured exec
    367us vs 439us for the R=5 + PE-transpose-epilogue baseline.
  - Min results land in bf16 staging [128 j, 256 rows]; the epilogue is
    PE-free: ACT double-tanh straight off staging, DMA out in transposed
    [jb, j, row] layout, and the host reassembles with cheap numpy
    transposes -- saving the PE transposes and freeing 2 PSUM banks.
"""

import sys

for _p in ("/opt/trn_rl_repo", "/root/.axon_site/_ro/trn_rl_repo"):
    if _p not in sys.path:
        sys.path.insert(0, _p)

import numpy as np

B, CIN, H, W = 64, 16, 256, 256
COUT, KK = 64, 3
HO, WO = H - 2, W - 2  # 254
N_CORES = 8
B_LOC = B // N_CORES  # 8 images per core

# geometry
WIN_ROWS = 6          # input rows per window
R = WIN_ROWS - KK + 1  # 4 output rows per window
KDIM = (CIN + 1) * WIN_ROWS  # 102 contraction rows (incl. ones channel)
NDIM = R * COUT       # 256 moving free size
MJ = 128              # output width positions per j-block
J0S = (0, WO - MJ)    # j origin per block; cols 126/127 overlap benignly
N_JB = 2
N_WIN = 64            # windows: row0 = 4w for w<63, 250 for w=63
N_PAIR = N_WIN // 2   # two windows share one PSUM bank
_cache = {}


def _row0(w):
    return 4 * w if w < N_WIN - 1 else HO - R  # 250


# Per-(image, jb) drain plan over the 32 bank-pairs, in bank order.
# 'A' banks: direct DVE reduce (~730ns measured).  'B' groups: each bank is
# ACT-copied to a shared SBUF buffer (~791ns ACT), then ONE flat 2x DVE
# fold + ONE reduce cover the whole group, amortizing DVE fixed costs
# (~425ns DVE per bank vs 644 ungrouped).  10 A + 22 B balances
# DVE ~266us vs ACT ~278us per core against the PE's 339us.
# Bank-tile t uses path B (ACT copy + flat DVE fold + reduce) when
# OFF_PAT[t % len] else path A (direct DVE reduce).  Measured per-bank:
# A = DVE ~730ns; B = ACT ~791ns + DVE ~644ns.  13/16 B balances
# DVE ~312us / ACT ~291us under the PE's ~339us.
OFF_PAT = (False, True, True, True, True, True, False, True, True, True,
           True, False, True, True, True, True)
OFF_PAT_12 = (False, True, True, True, False, True, True, True, False, True,
              True, True, False, True, True, True)
OFF_PAT_14 = (False, True, True, True, True, True, True, True, False, True,
              True, True, True, True, True, True)


def _build_wblocks(conv_weight, conv_bias):
    """wblk[dx][rho*17+ci, r*64+co] = W[co,ci,rho-r,dx]; bias on the ones-
    channel row (rho=0, ci=CIN) of dx=0.  Partition order matches the
    [B, H, C, W] host layout of x so the window DMA merges (row, chan)."""
    wblk = np.zeros((KK, KDIM, NDIM), dtype=np.float32)
    for dx in range(KK):
        for ci in range(CIN):
            for rho in range(WIN_ROWS):
                k = rho * (CIN + 1) + ci
                for r in range(R):
                    dy = rho - r
                    if 0 <= dy < KK:
                        wblk[dx, k, r * COUT:(r + 1) * COUT] = conv_weight[:, ci, dy, dx]
    k_bias = CIN  # (rho=0, ci=16)
    for r in range(R):
        wblk[0, k_bias, r * COUT:(r + 1) * COUT] = conv_bias
    return wblk


def _build_nc(reps=1, ablate=()):
    import concourse.bass as bass
    import concourse.bacc as bacc
    import concourse.tile as tile
    from concourse import mybir

    f32 = mybir.dt.float32
    bf16 = mybir.dt.bfloat16

    nc = bacc.Bacc(None)
    # x_aug host layout is [B, H, C, W]: window partitions are (row, chan)
    x_aug = nc.dram_tensor("x_aug", [B_LOC, H, CIN + 1, W], bf16, kind="ExternalInput")
    wblk_d = nc.dram_tensor("wblk", [KK, KDIM, NDIM], bf16, kind="ExternalInput")
    # output in transposed layout [img, jb, j, row]; host reassembles
    y = nc.dram_tensor("y", [B_LOC, N_JB, MJ, HO], f32, kind="ExternalOutput")

    with tile.TileContext(nc) as tc:
        with (
            tc.tile_pool(name="consts", bufs=1) as consts,
            tc.tile_pool(name="wins", bufs=3) as wins,
            tc.tile_pool(name="stage", bufs=4) as stage,
            tc.tile_pool(name="fold", bufs=2) as fold,
            tc.tile_pool(name="outs", bufs=4) as outs,
            tc.tile_pool(name="cpsum", bufs=8, space="PSUM") as cpsum,
        ):
            wblk_s = consts.tile([KDIM, KK, NDIM], bf16)
            nc.sync.dma_start(out=wblk_s[:], in_=wblk_d.rearrange("k d n -> d k n"))

            import contextlib
            loop_ctx = tc.For_i(0, reps, 1) if reps > 1 else contextlib.nullcontext()
            with loop_ctx:
                _emit_body(nc, tc, bass, mybir, ablate, locals())
    nc.finalize()
    return nc


def _emit_body(nc, tc, bass, mybir, ablate, env):
    f32 = env["f32"]
    bf16 = env["bf16"]
    x_aug, y = env["x_aug"], env["y"]
    wblk_s = env["wblk_s"]
    wins, stage, fold, outs = env["wins"], env["stage"], env["fold"], env["outs"]
    cpsum = env["cpsum"]
    CW = (CIN + 1) * W  # elements per image row (all channels)
    MIN = mybir.AluOpType.min

    def _stg_out(stg, p):
        """Staging view [2, 4] for pair p's 8 output rows.  Pairs 0..30 are
        contiguous (rows 8p..8p+7); the last pair overlaps benignly (rows
        248..251 and 250..253 -- row 250/251 written twice, same value)."""
        r0a, r0b = _row0(2 * p), _row0(2 * p + 1)
        return bass.AP(
            tensor=stg.tensor,
            offset=stg.offset + r0a,
            ap=[list(stg.ap[0]), [r0b - r0a, 2], [1, R]],
        )

    def _epilogue(b, stagings):
        for jb in range(N_JB):
            t1 = outs.tile([MJ, HO], f32, name="t1")
            nc.scalar.activation(
                out=t1[:], in_=stagings[jb][:, 0:HO],
                func=mybir.ActivationFunctionType.Tanh,
            )
            t2 = outs.tile([MJ, HO], f32, name="t2")
            nc.scalar.activation(
                out=t2[:], in_=t1[:],
                func=mybir.ActivationFunctionType.Tanh,
            )
            # out-DMA on the SP ring: a DMA occupies its issuing engine's
            # queue for the full transfer, and ACT is drain-critical
            nc.sync.dma_start(out=y[b, jb], in_=t2[:])

    for b in range(B_LOC):
        bigx = wins.tile([KDIM, N_WIN, W], bf16, name="bigx")
        # windows 0..62 (uniform row0 = 4w) in chunked DMAs on the SP ring
        # (ACT's ring would stall the drain copies); w=63 alone.  Image 0
        # only: a small 6-window first chunk so the first matmuls start
        # ~7us earlier (they otherwise gate on a 9us 1.1MB chunk after the
        # 7us NEFF preamble).
        x_b = x_aug[b]
        w_lo = 0
        for nw in ((6, 19, 19, 19) if b == 0 else (21, 21, 21)):
            src = bass.AP(
                tensor=x_b.tensor,
                offset=x_b.offset + 4 * w_lo * CW,
                ap=[[CW, WIN_ROWS], [W, CIN + 1], [4 * CW, nw], [1, W]],
            )
            nc.sync.dma_start(out=bigx[:, w_lo:w_lo + nw, :], in_=src)
            w_lo += nw
        nc.sync.dma_start(
            out=bigx[:, N_WIN - 1, :],
            in_=x_aug[b, HO - R:H, :, :].rearrange("r c w -> (r c) w"),
        )

        def _win(w):
            return bigx[:, w, :]

        stagings = []
        for jb in range(N_JB):
            staging = stage.tile([MJ, 256], bf16, name=f"staging{jb}", tag=f"st{jb}")
            stagings.append(staging)

        def _bank_matmuls(p, j0):
            psum = cpsum.tile([MJ, 2, NDIM], f32, name="psum")
            for u in range(2):
                win = _win(2 * p + u)
                for dx in range(KK):
                    nc.tensor.matmul(
                        out=psum[:, u],
                        lhsT=win[:, j0 + dx:j0 + dx + MJ],
                        rhs=wblk_s[:, dx, :],
                        start=(dx == 0),
                        stop=(dx == KK - 1),
                    )
            return psum

        pat = (OFF_PAT_12 if "x12" in ablate
               else OFF_PAT_14 if "x14" in ablate else OFF_PAT)
        for p in range(N_PAIR):
            for jb in range(N_JB):
                j0 = J0S[jb]
                t = p * N_JB + jb
                psum = _bank_matmuls(p, j0)
                offload = pat[t % len(pat)] and "nooff" not in ablate
                stg_view = _stg_out(stagings[jb], p)
                if offload:
                    # path B: ACT drains the bank as bf16 with cout-halves
                    # split to the outer axis; one flat 2x DVE fold then a
                    # half-size reduce
                    lb = fold.tile([MJ, 2, 2, R, 32], bf16, name="lb")
                    nc.scalar.activation(
                        out=lb.rearrange("p c2 u r c -> p u r c2 c"),
                        in_=psum.rearrange("p u (r c2 c) -> p u r c2 c",
                                           c2=2, c=32),
                        func=mybir.ActivationFunctionType.Copy,
                    )
                    lflat = lb.rearrange("p c2 u r c -> p (c2 u r c)")
                    g = fold.tile([MJ, 2, R, 32], bf16, name="g")
                    nc.vector.tensor_tensor(
                        out=g.rearrange("p u r c -> p (u r c)"),
                        in0=lflat[:, 0:2 * R * 32],
                        in1=lflat[:, 2 * R * 32:4 * R * 32],
                        op=MIN,
                    )
                    nc.vector.tensor_reduce(
                        out=stg_view,
                        in_=g[:],
                        axis=mybir.AxisListType.X,
                        op=MIN,
                    )
                else:
                    # path A: direct DVE reduce from the full PSUM bank
                    nc.vector.tensor_reduce(
                        out=stg_view,
                        in_=psum.rearrange("p u (r c) -> p u r c", c=COUT),
                        axis=mybir.AxisListType.X,
                        op=MIN,
                    )
        _epilogue(b, stagings)


def _get_compiled(reps=1, ablate=()):
    key = ("nc", reps, tuple(ablate))
    if key not in _cache:
        _cache[key] = _build_nc(reps, ablate)
    return _cache[key]


def _to_bf16(a):
    import ml_dtypes
    return np.asarray(a, dtype=np.float32).astype(ml_dtypes.bfloat16)


def make_in_maps(x, conv_weight, conv_bias):
    x = np.asarray(x, dtype=np.float32)
    x_aug = np.empty((B, H, CIN + 1, W), dtype=np.float32)
    x_aug[:, :, :CIN] = x.transpose(0, 2, 1, 3)
    x_aug[:, :, CIN] = 1.0
    x_aug = _to_bf16(x_aug)
    wblk = _to_bf16(_build_wblocks(
        np.asarray(conv_weight, dtype=np.float32),
        np.asarray(conv_bias, dtype=np.float32)))
    return [
        {
            "x_aug": np.ascontiguousarray(x_aug[c * B_LOC:(c + 1) * B_LOC]),
            "wblk": wblk,
        }
        for c in range(N_CORES)
    ]


def kernel(x, conv_weight, conv_bias):
    from concourse.bass_utils import run_bass_kernel_spmd

    nc = _get_compiled()
    in_maps = make_in_maps(x, conv_weight, conv_bias)
    res = run_bass_kernel_spmd(nc, in_maps, core_ids=list(range(N_CORES)))
    out = np.empty((B, 1, HO, WO), dtype=np.float32)
    for c in range(N_CORES):
        yc = res.results[c]["y"]  # [B_LOC, 2, MJ, HO]
        blk = out[c * B_LOC:(c + 1) * B_LOC, 0]
        blk[:, :, 0:MJ] = yc[:, 0].transpose(0, 2, 1)
        blk[:, :, J0S[1]:WO] = yc[:, 1].transpose(0, 2, 1)
    return out


# revision 28
# speedup vs baseline: 1.0157x; 1.0037x over previous
"""Trainium2 Bass kernel for: conv2d(16->64, 3x3, VALID) + bias -> min over
channels -> tanh(tanh()).  Input x [64,16,256,256] f32, output [64,1,254,254].

Strategy (per core, data-parallel over batch: 8 images/core):
  - Conv as matmuls with the x-patch (bf16) as the stationary operand and a
    block-Toeplitz weight matrix (bf16) moving: conv output lands as
    [width-positions (partitions), rows*couts (free)] in f32 PSUM and the
    channel-min is a free-dim reduce.  6-row windows give R=4 output rows
    per 3-matmul (dx) group; TWO consecutive windows pack into ONE 2KB PSUM
    bank ([128, 2, 4, 64] = 512 f32), so every drain op sees a full bank
    and the per-op fixed costs (120 cyc PSUM init on DVE, 172 on ACT)
    amortize over 8 output rows instead of 5.
  - The channel-min drains 33M f32 PSUM elements/core -- more than either
    DVE or ACT alone can move at the PE's pace -- so bank-tiles alternate
    between two pipelines: path A = direct DVE tensor_reduce (~730ns);
    path B = ACT copy PSUM->SBUF bf16 in a cout-half-split layout (~790ns
    ACT), one FLAT single-dim-AP DVE tensor_tensor min fold (flat APs are
    required for the 2x_1p uop to engage; strided views run 1x) and a
    half-size DVE reduce (~640ns DVE total).  OFF_PAT at 13/16 B balances
    DVE ~312us and ACT ~291us under the PE's ~339us; measured exec
    367us vs 439us for the R=5 + PE-transpose-epilogue baseline.
  - Min results land in bf16 staging [128 j, 256 rows]; the epilogue is
    PE-free: ACT double-tanh straight off staging, DMA out in transposed
    [jb, j, row] layout, and the host reassembles with cheap numpy
    transposes -- saving the PE transposes and freeing 2 PSUM banks.
"""

import sys

for _p in ("/opt/trn_rl_repo", "/root/.axon_site/_ro/trn_rl_repo"):
    if _p not in sys.path:
        sys.path.insert(0, _p)

import numpy as np

B, CIN, H, W = 64, 16, 256, 256
COUT, KK = 64, 3
HO, WO = H - 2, W - 2  # 254
N_CORES = 8
B_LOC = B // N_CORES  # 8 images per core

# geometry
WIN_ROWS = 6          # input rows per window
R = WIN_ROWS - KK + 1  # 4 output rows per window
KDIM = (CIN + 1) * WIN_ROWS  # 102 contraction rows (incl. ones channel)
NDIM = R * COUT       # 256 moving free size
MJ = 128              # output width positions per j-block
J0S = (0, WO - MJ)    # j origin per block; cols 126/127 overlap benignly
N_JB = 2
N_WIN = 64            # windows: row0 = 4w for w<63, 250 for w=63
N_PAIR = N_WIN // 2   # two windows share one PSUM bank
_cache = {}


def _row0(w):
    return 4 * w if w < N_WIN - 1 else HO - R  # 250


# Per-(image, jb) drain plan over the 32 bank-pairs, in bank order.
# 'A' banks: direct DVE reduce (~730ns measured).  'B' groups: each bank is
# ACT-copied to a shared SBUF buffer (~791ns ACT), then ONE flat 2x DVE
# fold + ONE reduce cover the whole group, amortizing DVE fixed costs
# (~425ns DVE per bank vs 644 ungrouped).  10 A + 22 B balances
# DVE ~266us vs ACT ~278us per core against the PE's 339us.
# Bank-tile t uses path B (ACT copy + flat DVE fold + reduce) when
# OFF_PAT[t % len] else path A (direct DVE reduce).  Measured per-bank:
# A = DVE ~730ns; B = ACT ~791ns + DVE ~644ns.  13/16 B balances
# DVE ~312us / ACT ~291us under the PE's ~339us.
OFF_PAT = (False, True, True, True, True, True, False, True, True, True,
           True, False, True, True, True, True)
OFF_PAT_12 = (False, True, True, True, False, True, True, True, False, True,
              True, True, False, True, True, True)
OFF_PAT_14 = (False, True, True, True, True, True, True, True, False, True,
              True, True, True, True, True, True)


def _build_wblocks(conv_weight, conv_bias):
    """wblk[dx][rho*17+ci, r*64+co] = W[co,ci,rho-r,dx]; bias on the ones-
    channel row (rho=0, ci=CIN) of dx=0.  Partition order matches the
    [B, H, C, W] host layout of x so the window DMA merges (row, chan)."""
    wblk = np.zeros((KK, KDIM, NDIM), dtype=np.float32)
    for dx in range(KK):
        for ci in range(CIN):
            for rho in range(WIN_ROWS):
                k = rho * (CIN + 1) + ci
                for r in range(R):
                    dy = rho - r
                    if 0 <= dy < KK:
                        wblk[dx, k, r * COUT:(r + 1) * COUT] = conv_weight[:, ci, dy, dx]
    k_bias = CIN  # (rho=0, ci=16)
    for r in range(R):
        wblk[0, k_bias, r * COUT:(r + 1) * COUT] = conv_bias
    return wblk


def _build_nc(reps=1, ablate=()):
    import concourse.bass as bass
    import concourse.bacc as bacc
    import concourse.tile as tile
    from concourse import mybir

    f32 = mybir.dt.float32
    bf16 = mybir.dt.bfloat16

    nc = bacc.Bacc(None)
    # x_aug host layout is [B, H, C, W]: window partitions are (row, chan)
    x_aug = nc.dram_tensor("x_aug", [B_LOC, H, CIN + 1, W], bf16, kind="ExternalInput")
    wblk_d = nc.dram_tensor("wblk", [KK, KDIM, NDIM], bf16, kind="ExternalInput")
    # output in transposed layout [img, jb, j, row]; host reassembles
    y = nc.dram_tensor("y", [B_LOC, N_JB, MJ, HO], f32, kind="ExternalOutput")

    with tile.TileContext(nc) as tc:
        with (
            tc.tile_pool(name="consts", bufs=1) as consts,
            tc.tile_pool(name="wins", bufs=3) as wins,
            tc.tile_pool(name="stage", bufs=4) as stage,
            tc.tile_pool(name="fold", bufs=2) as fold,
            tc.tile_pool(name="outs", bufs=4) as outs,
            tc.tile_pool(name="cpsum", bufs=8, space="PSUM") as cpsum,
        ):
            wblk_s = consts.tile([KDIM, KK, NDIM], bf16)
            nc.sync.dma_start(out=wblk_s[:], in_=wblk_d.rearrange("k d n -> d k n"))

            import contextlib
            loop_ctx = tc.For_i(0, reps, 1) if reps > 1 else contextlib.nullcontext()
            with loop_ctx:
                _emit_body(nc, tc, bass, mybir, ablate, locals())
    nc.finalize()
    return nc


def _emit_body(nc, tc, bass, mybir, ablate, env):
    f32 = env["f32"]
    bf16 = env["bf16"]
    x_aug, y = env["x_aug"], env["y"]
    wblk_s = env["wblk_s"]
    wins, stage, fold, outs = env["wins"], env["stage"], env["fold"], env["outs"]
    cpsum = env["cpsum"]
    CW = (CIN + 1) * W  # elements per image row (all channels)
    MIN = mybir.AluOpType.min

    def _stg_out(stg, p):
        """Staging view [2, 4] for pair p's 8 output rows.  Pairs 0..30 are
        contiguous (rows 8p..8p+7); the last pair overlaps benignly (rows
        248..251 and 250..253 -- row 250/251 written twice, same value)."""
        r0a, r0b = _row0(2 * p), _row0(2 * p + 1)
        return bass.AP(
            tensor=stg.tensor,
            offset=stg.offset + r0a,
            ap=[list(stg.ap[0]), [r0b - r0a, 2], [1, R]],
        )

    def _epilogue(b, stagings):
        for jb in range(N_JB):
            t1 = outs.tile([MJ, HO], f32, name="t1")
            nc.scalar.activation(
                out=t1[:], in_=stagings[jb][:, 0:HO],
                func=mybir.ActivationFunctionType.Tanh,
            )
            t2 = outs.tile([MJ, HO], f32, name="t2")
            nc.scalar.activation(
                out=t2[:], in_=t1[:],
                func=mybir.ActivationFunctionType.Tanh,
            )
            # out-DMA on the SP ring: a DMA occupies its issuing engine's
            # queue for the full transfer, and ACT is drain-critical
            nc.sync.dma_start(out=y[b, jb], in_=t2[:])

    for b in range(B_LOC):
        bigx = wins.tile([KDIM, N_WIN, W], bf16, name="bigx")
        # windows 0..62 (uniform row0 = 4w) in chunked DMAs on the SP ring
        # (ACT's ring would stall the drain copies); w=63 alone.  Image 0
        # only: a small 6-window first chunk so the first matmuls start
        # ~7us earlier (they otherwise gate on a 9us 1.1MB chunk after the
        # 7us NEFF preamble).
        x_b = x_aug[b]
        w_lo = 0
        for nw in ((10, 18, 18, 17) if b == 0 else (21, 21, 21)):
            src = bass.AP(
                tensor=x_b.tensor,
                offset=x_b.offset + 4 * w_lo * CW,
                ap=[[CW, WIN_ROWS], [W, CIN + 1], [4 * CW, nw], [1, W]],
            )
            nc.sync.dma_start(out=bigx[:, w_lo:w_lo + nw, :], in_=src)
            w_lo += nw
        nc.sync.dma_start(
            out=bigx[:, N_WIN - 1, :],
            in_=x_aug[b, HO - R:H, :, :].rearrange("r c w -> (r c) w"),
        )

        def _win(w):
            return bigx[:, w, :]

        stagings = []
        for jb in range(N_JB):
            staging = stage.tile([MJ, 256], bf16, name=f"staging{jb}", tag=f"st{jb}")
            stagings.append(staging)

        def _bank_matmuls(p, j0):
            psum = cpsum.tile([MJ, 2, NDIM], f32, name="psum")
            for u in range(2):
                win = _win(2 * p + u)
                for dx in range(KK):
                    nc.tensor.matmul(
                        out=psum[:, u],
                        lhsT=win[:, j0 + dx:j0 + dx + MJ],
                        rhs=wblk_s[:, dx, :],
                        start=(dx == 0),
                        stop=(dx == KK - 1),
                    )
            return psum

        pat = (OFF_PAT_12 if "x12" in ablate
               else OFF_PAT_14 if "x14" in ablate else OFF_PAT)
        for p in range(N_PAIR):
            for jb in range(N_JB):
                j0 = J0S[jb]
                t = p * N_JB + jb
                psum = _bank_matmuls(p, j0)
                # last image's final banks take the low-latency direct
                # path so the drain tail after the final matmul is short
                tail_a = (b == B_LOC - 1 and p >= N_PAIR - 4)
                offload = (pat[t % len(pat)] and not tail_a
                           and "nooff" not in ablate)
                stg_view = _stg_out(stagings[jb], p)
                if offload:
                    # path B: ACT drains the bank as bf16 with cout-halves
                    # split to the outer axis; one flat 2x DVE fold then a
                    # half-size reduce
                    lb = fold.tile([MJ, 2, 2, R, 32], bf16, name="lb")
                    nc.scalar.activation(
                        out=lb.rearrange("p c2 u r c -> p u r c2 c"),
                        in_=psum.rearrange("p u (r c2 c) -> p u r c2 c",
                                           c2=2, c=32),
                        func=mybir.ActivationFunctionType.Copy,
                    )
                    lflat = lb.rearrange("p c2 u r c -> p (c2 u r c)")
                    g = fold.tile([MJ, 2, R, 32], bf16, name="g")
                    nc.vector.tensor_tensor(
                        out=g.rearrange("p u r c -> p (u r c)"),
                        in0=lflat[:, 0:2 * R * 32],
                        in1=lflat[:, 2 * R * 32:4 * R * 32],
                        op=MIN,
                    )
                    nc.vector.tensor_reduce(
                        out=stg_view,
                        in_=g[:],
                        axis=mybir.AxisListType.X,
                        op=MIN,
                    )
                else:
                    # path A: direct DVE reduce from the full PSUM bank
                    nc.vector.tensor_reduce(
                        out=stg_view,
                        in_=psum.rearrange("p u (r c) -> p u r c", c=COUT),
                        axis=mybir.AxisListType.X,
                        op=MIN,
                    )
        _epilogue(b, stagings)


def _get_compiled(reps=1, ablate=()):
    key = ("nc", reps, tuple(ablate))
    if key not in _cache:
        _cache[key] = _build_nc(reps, ablate)
    return _cache[key]


def _to_bf16(a):
    import ml_dtypes
    return np.asarray(a, dtype=np.float32).astype(ml_dtypes.bfloat16)


def make_in_maps(x, conv_weight, conv_bias):
    x = np.asarray(x, dtype=np.float32)
    x_aug = np.empty((B, H, CIN + 1, W), dtype=np.float32)
    x_aug[:, :, :CIN] = x.transpose(0, 2, 1, 3)
    x_aug[:, :, CIN] = 1.0
    x_aug = _to_bf16(x_aug)
    wblk = _to_bf16(_build_wblocks(
        np.asarray(conv_weight, dtype=np.float32),
        np.asarray(conv_bias, dtype=np.float32)))
    return [
        {
            "x_aug": np.ascontiguousarray(x_aug[c * B_LOC:(c + 1) * B_LOC]),
            "wblk": wblk,
        }
        for c in range(N_CORES)
    ]


def kernel(x, conv_weight, conv_bias):
    from concourse.bass_utils import run_bass_kernel_spmd

    nc = _get_compiled()
    in_maps = make_in_maps(x, conv_weight, conv_bias)
    res = run_bass_kernel_spmd(nc, in_maps, core_ids=list(range(N_CORES)))
    out = np.empty((B, 1, HO, WO), dtype=np.float32)
    for c in range(N_CORES):
        yc = res.results[c]["y"]  # [B_LOC, 2, MJ, HO]
        blk = out[c * B_LOC:(c + 1) * B_LOC, 0]
        blk[:, :, 0:MJ] = yc[:, 0].transpose(0, 2, 1)
        blk[:, :, J0S[1]:WO] = yc[:, 1].transpose(0, 2, 1)
    return out
